# revision 13
# baseline (speedup 1.0000x reference)
"""Trainium2 Bass kernel for CachedLightningIndexer-style scoring.

Reference computation (b=2, t=s=4096, d_model=2048, heads=2, dim=32):
    q = (x @ wq).reshape(b, t, 2, 32); k = x @ wk; w = x @ ww
    scores[b,t,s] = sum_h w[b,t,h] * relu(q[b,t,h,:] . k[b,s,:])

Sharding (8 cores): output grid of 4 t-quarters x 2 s-halves over the
flattened (b*t) = 8192 rows. Core (i, j) computes scores for t rows
[2048*i, 2048*(i+1)) against s columns [2048*j, 2048*(j+1)) of batch
b = i // 2. Everything is computed on-device from x; the host only
reshapes/transposes/casts inputs and concatenates outputs.

Per-core pipeline, pipelined over t-halves (1024 rows) and s-chunks
(1024 cols) so compute starts after only ~8.4MB of input lands:
  load xt-half0 + xs-chunk0 -> project q|w (half0) and k (chunk0)
  -> score block (t-half0 x s-chunk0) while xt-half1 streams
  -> project half1 -> block (h1 x c0) -> keys chunk1 -> blocks (c1).
Dots: four K=32 matmuls per position (2 heads x 2 col-halves) rotated
across the four 32-row PE tile groups so they execute concurrently
(qT4 rows [q0,q0,q1,q1], kT4 = k duplicated to all four groups).
Scoring with per-partition r = w1/w0 (clamped):
  plan A: ACT r0=relu(d0); DVE r1=max(d1,0)*r; v=r0+r1; out=v*w0
  plan B: ACT r0=relu(d0), r1u=relu(d1); DVE r1=r1u*r; v=r0+r1; out=v*w0
(plan B on ~60% of tiles balances ACT vs DVE load)
"""

import numpy as np
import ml_dtypes

import concourse.bass as bass
import concourse.mybir as mybir
import concourse.tile as tile
from concourse import bacc
from concourse.bass_utils import run_bass_kernel_spmd
from concourse.masks import make_identity

BF16 = ml_dtypes.bfloat16

D_MODEL = 2048
B = 2
T = 4096
DIM = 32
N_CORES = 8
Q = 2048
S = 2048
KT = D_MODEL // 128   # 16
NJ = Q // 128         # 16
NC = 2
SC = S // NC          # 1024

_cached = {}


def _build():
    out_dt = mybir.dt.bfloat16
    nc = bacc.Bacc("TRN2", target_bir_lowering=False, debug=False,
                   num_devices=N_CORES)
    xTt = nc.dram_tensor("xTt", [128, KT, Q], mybir.dt.bfloat16,
                         kind="ExternalInput").ap()
    xTs = nc.dram_tensor("xTs", [128, KT, S], mybir.dt.bfloat16,
                         kind="ExternalInput").ap()
    wqw = nc.dram_tensor("wqw", [128, KT, 66], mybir.dt.bfloat16,
                         kind="ExternalInput").ap()
    wkk = nc.dram_tensor("wkk", [128, KT, DIM], mybir.dt.bfloat16,
                         kind="ExternalInput").ap()
    out = nc.dram_tensor("out", [Q, S], out_dt, kind="ExternalOutput").ap()

    f32 = mybir.dt.float32
    bf16 = mybir.dt.bfloat16
    Alu = mybir.AluOpType
    Act = mybir.ActivationFunctionType

    with tile.TileContext(nc) as tc:
        with tc.tile_pool(name="wpool", bufs=1) as wpool, \
             tc.tile_pool(name="xpool", bufs=1) as xpool, \
             tc.tile_pool(name="xs0pool", bufs=KT) as xs0pool, \
             tc.tile_pool(name="xs1pool", bufs=4) as xs1pool, \
             tc.tile_pool(name="spool", bufs=1) as spool:

            wqw_sb = wpool.tile([128, KT, 66], bf16)
            nc.sync.dma_start(wqw_sb[:], wqw[:])
            wk_sb = wpool.tile([128, KT, DIM], bf16)
            nc.sync.dma_start(wk_sb[:], wkk[:])
            ident2 = wpool.tile([2, 2], f32)
            make_identity(nc, ident2[:])

            # x^T slab: t-half0 first (with chunk-0 keys), half1 after
            xt_all = xpool.tile([128, KT, Q], bf16)
            for i in range(4):
                nc.sync.dma_start(xt_all[:, i * 4:(i + 1) * 4, 0:SC],
                                  xTt[:, i * 4:(i + 1) * 4, 0:SC])
            xs0 = []
            for kt in range(KT):
                t_ = xs0pool.tile([128, SC], bf16, tag="xs0", name=f"xs0_{kt}")
                nc.sync.dma_start(t_[:], xTs[:, kt, 0:SC])
                xs0.append(t_)
            for i in range(4):
                nc.sync.dma_start(xt_all[:, i * 4:(i + 1) * 4, SC:2 * SC],
                                  xTt[:, i * 4:(i + 1) * 4, SC:2 * SC])

            # rot4 operand layouts
            qT4 = spool.tile([128, Q], bf16)
            kT4 = spool.tile([128, S], bf16)
            wT_sb = spool.tile([2, Q], f32)
            wvec = spool.tile([128, NJ, 2], f32)
            rvec = spool.tile([128, NJ], f32)
            w0recip = spool.tile([128, NJ], f32)

            with tc.tile_pool(name="psum", bufs=4, space="PSUM") as dpool:
                # PE warm-up: trip the HAM clock gate before real work
                ps_wu = dpool.tile([128, SC], f32, tag="d")
                for _ in range(10):
                    nc.tensor.matmul(ps_wu[0:32, 0:462], lhsT=wk_sb[:, 0, :],
                                     rhs=wqw_sb[:, 0:7, :])

                def proj_half(h):
                    ps_qw = dpool.tile([128, SC], f32, tag="d", name=f"ps_qw{h}")
                    hsl = slice(h * SC, (h + 1) * SC)
                    for kt in range(KT):
                        st, sp = (kt == 0), (kt == KT - 1)
                        for n in range(SC // 512):
                            sl = slice(n * 512, (n + 1) * 512)
                            gsl = slice(h * SC + n * 512, h * SC + (n + 1) * 512)
                            nc.tensor.matmul(ps_qw[0:66, sl], lhsT=wqw_sb[:, kt, :],
                                             rhs=xt_all[:, kt, gsl],
                                             start=st, stop=sp)
                    nc.scalar.copy(qT4[0:32, hsl], ps_qw[0:32, :])
                    nc.scalar.copy(qT4[64:96, hsl], ps_qw[32:64, :])
                    nc.vector.tensor_copy(qT4[32:64, hsl], qT4[0:32, hsl])
                    nc.vector.tensor_copy(qT4[96:128, hsl], qT4[64:96, hsl])
                    nc.vector.tensor_copy(wT_sb[:, hsl], ps_qw[64:66, :])

                def keys_chunk(c, xs_tiles):
                    ps_kx = dpool.tile([128, SC], f32, tag="d", name=f"ps_k{c}")
                    csl = slice(c * SC, (c + 1) * SC)
                    for kt in range(KT):
                        st, sp = (kt == 0), (kt == KT - 1)
                        for n in range(SC // 512):
                            sl = slice(n * 512, (n + 1) * 512)
                            nc.tensor.matmul(ps_kx[0:32, sl],
                                             lhsT=wk_sb[:, kt, :],
                                             rhs=xs_tiles[kt][:, sl],
                                             start=st, stop=sp)
                    nc.scalar.copy(kT4[0:32, csl], ps_kx[0:32, :])
                    for g in range(1, 4):
                        nc.vector.tensor_copy(kT4[32 * g:32 * (g + 1), csl],
                                              kT4[0:32, csl])

                def weights_half(h):
                    # transpose [2,128] -> [128,2] per t-tile, batched
                    ps_kx = dpool.tile([128, SC], f32, tag="d", name=f"ps_w{h}")
                    jlo, jhi = h * (NJ // 2), (h + 1) * (NJ // 2)
                    for jj in range(jlo, jhi):
                        nc.tensor.transpose(
                            ps_kx[:, 2 * jj:2 * jj + 2],
                            wT_sb[:, jj * 128:(jj + 1) * 128], ident2[:])
                    nc.scalar.copy(wvec[:, jlo:jhi, :],
                                   ps_kx[:, 2 * jlo:2 * jhi])
                    nc.vector.reciprocal(w0recip[:, jlo:jhi],
                                         wvec[:, jlo:jhi, 0])
                    nc.vector.tensor_tensor(rvec[:, jlo:jhi],
                                            wvec[:, jlo:jhi, 1],
                                            w0recip[:, jlo:jhi], Alu.mult)
                    nc.vector.tensor_scalar(rvec[:, jlo:jhi], rvec[:, jlo:jhi],
                                            1e20, -1e20, Alu.min, Alu.max)

                proj_half(0)
                keys_chunk(0, xs0)
                weights_half(0)

                with tc.tile_pool(name="rpool", bufs=4) as rpool, \
                     tc.tile_pool(name="opool", bufs=4) as opool:

                    def block(c, jlo, jhi):
                        csl = slice(c * SC, (c + 1) * SC)
                        for jj in range(jlo, jhi):
                            pos = c * NJ + jj
                            tsl = slice(jj * 128, (jj + 1) * 128)
                            w0 = wvec[:, jj, 0:1]
                            rv = rvec[:, jj:jj + 1]
                            plan_b = pos % 3 != 0   # ~2/3 plan B
                            d0 = dpool.tile([128, SC], f32, tag="d")
                            d1 = dpool.tile([128, SC], f32, tag="d")
                            for g, (dd, n) in enumerate(
                                    ((d0, 0), (d0, 1), (d1, 0), (d1, 1))):
                                qrow = slice(32 * g, 32 * (g + 1))
                                sl = slice(n * 512, (n + 1) * 512)
                                ksl = slice(c * SC + n * 512,
                                            c * SC + (n + 1) * 512)
                                nc.tensor.matmul(dd[:, sl], lhsT=qT4[qrow, tsl],
                                                 rhs=kT4[qrow, ksl],
                                                 tile_position=(32 * g, 0))
                            r0 = rpool.tile([128, SC], bf16, tag="r0")
                            nc.scalar.activation(r0[:], d0[:], Act.Relu)
                            r1 = rpool.tile([128, SC], bf16, tag="r1")
                            if plan_b:
                                r1u = rpool.tile([128, SC], bf16, tag="r1u")
                                nc.scalar.activation(r1u[:], d1[:], Act.Relu)
                                nc.vector.tensor_scalar(r1[:], r1u[:], rv, None,
                                                        Alu.mult)
                            else:
                                nc.vector.tensor_scalar(r1[:], d1[:], 0.0, rv,
                                                        Alu.max, Alu.mult)
                            v = rpool.tile([128, SC], bf16, tag="v")
                            nc.vector.tensor_tensor(v[:], r0[:], r1[:], Alu.add)
                            ot = opool.tile([128, SC], out_dt, tag="ot")
                            nc.vector.tensor_scalar(ot[:], v[:], w0, None,
                                                    Alu.mult)
                            nc.sync.dma_start(out[tsl, csl], ot[:])

                    block(0, 0, NJ // 2)
                    proj_half(1)
                    weights_half(1)
                    block(0, NJ // 2, NJ)

                    # chunk-1 keys: contiguous PE block between chunks
                    xs1 = []
                    for kt in range(KT):
                        xs = xs1pool.tile([128, SC], bf16, tag="xs1")
                        nc.sync.dma_start(xs[:], xTs[:, kt, SC:2 * SC])
                        xs1.append(xs)
                    keys_chunk(1, xs1)

                    block(1, 0, NJ)
    nc.compile()
    return nc


def _get_nc():
    if "nc" not in _cached:
        _cached["nc"] = _build()
    return _cached["nc"]


def _make_in_maps(x, wq, wk, ww):
    x_flat = np.asarray(x, dtype=np.float32).reshape(B * T, D_MODEL)
    xT = x_flat.T.astype(BF16)                       # [2048, 8192]
    xTr = np.ascontiguousarray(                      # [128, 16, 8192]
        xT.reshape(KT, 128, B * T).transpose(1, 0, 2))
    wqw = np.ascontiguousarray(
        np.concatenate([np.asarray(wq), np.asarray(ww)], axis=1).astype(BF16)
        .reshape(KT, 128, 66).transpose(1, 0, 2))
    wkk = np.ascontiguousarray(
        np.asarray(wk).astype(BF16).reshape(KT, 128, DIM).transpose(1, 0, 2))
    in_maps = []
    for core in range(N_CORES):
        i, j = core // 2, core % 2
        b = i // 2
        t_lo = i * Q
        s_lo = b * T + j * S
        in_maps.append({
            "xTt": np.ascontiguousarray(xTr[:, :, t_lo:t_lo + Q]),
            "xTs": np.ascontiguousarray(xTr[:, :, s_lo:s_lo + S]),
            "wqw": wqw,
            "wkk": wkk,
        })
    return in_maps


def run(x, wq, wk, ww, trace=False, **kw):
    nc = _get_nc()
    in_maps = _make_in_maps(x, wq, wk, ww)
    res = run_bass_kernel_spmd(nc, in_maps, list(range(N_CORES)),
                               trace=trace, **kw)
    out = np.empty((B * T, T), dtype=np.float32)
    for core in range(N_CORES):
        i, j = core // 2, core % 2
        blk = res.results[core]["out"]
        out[i * Q:(i + 1) * Q, j * S:(j + 1) * S] = blk.astype(np.float32)
    return out.reshape(B, T, T), res


def kernel(x, wq, wk, ww):
    out, _ = run(x, wq, wk, ww, trace=False)
    return out


# revision 14
# speedup vs baseline: 1.1611x; 1.1611x over previous
"""Trainium2 Bass kernel for CachedLightningIndexer-style scoring.

Reference computation (b=2, t=s=4096, d_model=2048, heads=2, dim=32):
    q = (x @ wq).reshape(b, t, 2, 32); k = x @ wk; w = x @ ww
    scores[b,t,s] = sum_h w[b,t,h] * relu(q[b,t,h,:] . k[b,s,:])

Sharding (8 cores): output grid of 4 t-quarters x 2 s-halves over the
flattened (b*t) = 8192 rows. Core (i, j) computes scores for t rows
[2048*i, 2048*(i+1)) against s columns [2048*j, 2048*(j+1)) of batch
b = i // 2. Everything is computed on-device from x; the host only
reshapes/transposes/casts inputs and concatenates outputs.

Per-core pipeline, pipelined over t-halves (1024 rows) and s-chunks
(1024 cols) so compute starts after only ~8.4MB of input lands:
  load xt-half0 + xs-chunk0 -> project q|w (half0) and k (chunk0)
  -> score block (t-half0 x s-chunk0) while xt-half1 streams
  -> project half1 -> block (h1 x c0) -> keys chunk1 -> blocks (c1).
Dots: four K=32 matmuls per position (2 heads x 2 col-halves) rotated
across the four 32-row PE tile groups so they execute concurrently
(qT4 rows [q0,q0,q1,q1], kT4 = k duplicated to all four groups).
Scoring with per-partition r = w1/w0 (clamped):
  plan A: ACT r0=relu(d0); DVE r1=max(d1,0)*r; v=r0+r1; out=v*w0
  plan B: ACT r0=relu(d0), r1u=relu(d1); DVE r1=r1u*r; v=r0+r1; out=v*w0
(plan B on ~60% of tiles balances ACT vs DVE load)
"""

import numpy as np
import ml_dtypes

import concourse.bass as bass
import concourse.mybir as mybir
import concourse.tile as tile
from concourse import bacc
from concourse.bass_utils import run_bass_kernel_spmd
from concourse.masks import make_identity

BF16 = ml_dtypes.bfloat16

D_MODEL = 2048
B = 2
T = 4096
DIM = 32
N_CORES = 8
Q = 2048
S = 2048
KT = D_MODEL // 128   # 16
NJ = Q // 128         # 16
NC = 2
SC = S // NC          # 1024

_cached = {}


def _build():
    out_dt = mybir.dt.bfloat16
    nc = bacc.Bacc("TRN2", target_bir_lowering=False, debug=False,
                   num_devices=N_CORES)
    xTt = nc.dram_tensor("xTt", [128, KT, Q], mybir.dt.bfloat16,
                         kind="ExternalInput").ap()
    xTs = nc.dram_tensor("xTs", [128, KT, S], mybir.dt.bfloat16,
                         kind="ExternalInput").ap()
    wqw = nc.dram_tensor("wqw", [128, KT, 66], mybir.dt.bfloat16,
                         kind="ExternalInput").ap()
    wkk = nc.dram_tensor("wkk", [128, KT, DIM], mybir.dt.bfloat16,
                         kind="ExternalInput").ap()
    out = nc.dram_tensor("out", [Q, S], out_dt, kind="ExternalOutput").ap()

    f32 = mybir.dt.float32
    bf16 = mybir.dt.bfloat16
    Alu = mybir.AluOpType
    Act = mybir.ActivationFunctionType

    with tile.TileContext(nc) as tc:
        with tc.tile_pool(name="wpool", bufs=1) as wpool, \
             tc.tile_pool(name="xpool", bufs=1) as xpool, \
             tc.tile_pool(name="xs0pool", bufs=KT) as xs0pool, \
             tc.tile_pool(name="xs1pool", bufs=4) as xs1pool, \
             tc.tile_pool(name="spool", bufs=1) as spool:

            wqw_sb = wpool.tile([128, KT, 66], bf16)
            nc.sync.dma_start(wqw_sb[:], wqw[:])
            wk_sb = wpool.tile([128, KT, DIM], bf16)
            nc.sync.dma_start(wk_sb[:], wkk[:])
            ident2 = wpool.tile([2, 2], f32)
            make_identity(nc, ident2[:])

            # x^T slab: t-half0 first (with chunk-0 keys), half1 after
            xt_all = xpool.tile([128, KT, Q], bf16)
            for i in range(4):
                nc.sync.dma_start(xt_all[:, i * 4:(i + 1) * 4, 0:SC],
                                  xTt[:, i * 4:(i + 1) * 4, 0:SC])
            xs0 = []
            for kt in range(KT):
                t_ = xs0pool.tile([128, SC], bf16, tag="xs0", name=f"xs0_{kt}")
                nc.sync.dma_start(t_[:], xTs[:, kt, 0:SC])
                xs0.append(t_)
            for i in range(4):
                nc.sync.dma_start(xt_all[:, i * 4:(i + 1) * 4, SC:2 * SC],
                                  xTt[:, i * 4:(i + 1) * 4, SC:2 * SC])

            # rot4 operand layouts
            qT4 = spool.tile([128, Q], bf16)
            kT4 = spool.tile([128, S], bf16)
            wT_sb = spool.tile([2, Q], f32)
            wvec = spool.tile([128, NJ, 2], f32)
            rvec = spool.tile([128, NJ], f32)
            w0recip = spool.tile([128, NJ], f32)

            with tc.tile_pool(name="psK", bufs=1, space="PSUM") as psK, \
                 tc.tile_pool(name="psA", bufs=1, space="PSUM") as psA:
                # ps_kx rows 0-31: key projection; full 128 rows reused as
                # scratch for the batched weight transposes between phases
                ps_kx = psK.tile([128, SC], f32)
                ps_qw = psA.tile([66, SC], f32)

                # PE warm-up: trip the HAM clock gate before real work
                for _ in range(10):
                    nc.tensor.matmul(ps_qw[0:32, 0:462], lhsT=wk_sb[:, 0, :],
                                     rhs=wqw_sb[:, 0:7, :])

                def proj_half(h):
                    hsl = slice(h * SC, (h + 1) * SC)
                    for kt in range(KT):
                        st, sp = (kt == 0), (kt == KT - 1)
                        for n in range(SC // 512):
                            sl = slice(n * 512, (n + 1) * 512)
                            gsl = slice(h * SC + n * 512, h * SC + (n + 1) * 512)
                            nc.tensor.matmul(ps_qw[:, sl], lhsT=wqw_sb[:, kt, :],
                                             rhs=xt_all[:, kt, gsl],
                                             start=st, stop=sp)
                    nc.scalar.copy(qT4[0:32, hsl], ps_qw[0:32, :])
                    nc.scalar.copy(qT4[64:96, hsl], ps_qw[32:64, :])
                    nc.vector.tensor_copy(qT4[32:64, hsl], qT4[0:32, hsl])
                    nc.vector.tensor_copy(qT4[96:128, hsl], qT4[64:96, hsl])
                    nc.vector.tensor_copy(wT_sb[:, hsl], ps_qw[64:66, :])

                def keys_chunk(c, xs_tiles):
                    csl = slice(c * SC, (c + 1) * SC)
                    for kt in range(KT):
                        st, sp = (kt == 0), (kt == KT - 1)
                        for n in range(SC // 512):
                            sl = slice(n * 512, (n + 1) * 512)
                            nc.tensor.matmul(ps_kx[0:32, sl],
                                             lhsT=wk_sb[:, kt, :],
                                             rhs=xs_tiles[kt][:, sl],
                                             start=st, stop=sp)
                    nc.scalar.copy(kT4[0:32, csl], ps_kx[0:32, :])
                    for g in range(1, 4):
                        nc.vector.tensor_copy(kT4[32 * g:32 * (g + 1), csl],
                                              kT4[0:32, csl])

                def weights_half(h):
                    # transpose [2,128] -> [128,2] per t-tile, batched
                    jlo, jhi = h * (NJ // 2), (h + 1) * (NJ // 2)
                    for jj in range(jlo, jhi):
                        nc.tensor.transpose(
                            ps_kx[:, 2 * jj:2 * jj + 2],
                            wT_sb[:, jj * 128:(jj + 1) * 128], ident2[:])
                    nc.scalar.copy(wvec[:, jlo:jhi, :],
                                   ps_kx[:, 2 * jlo:2 * jhi])
                    nc.vector.reciprocal(w0recip[:, jlo:jhi],
                                         wvec[:, jlo:jhi, 0])
                    nc.vector.tensor_tensor(rvec[:, jlo:jhi],
                                            wvec[:, jlo:jhi, 1],
                                            w0recip[:, jlo:jhi], Alu.mult)
                    nc.vector.tensor_scalar(rvec[:, jlo:jhi], rvec[:, jlo:jhi],
                                            1e20, -1e20, Alu.min, Alu.max)

                proj_half(0)
                keys_chunk(0, xs0)
                weights_half(0)

                with tc.tile_pool(name="psumd", bufs=2, space="PSUM") as dpool, \
                     tc.tile_pool(name="rpool", bufs=4) as rpool, \
                     tc.tile_pool(name="opool", bufs=4) as opool:

                    def block(c, jlo, jhi):
                        csl = slice(c * SC, (c + 1) * SC)
                        for jj in range(jlo, jhi):
                            pos = c * NJ + jj
                            tsl = slice(jj * 128, (jj + 1) * 128)
                            w0 = wvec[:, jj, 0:1]
                            rv = rvec[:, jj:jj + 1]
                            plan_b = pos % 3 != 0   # ~2/3 plan B
                            d0 = dpool.tile([128, SC], f32, tag="d")
                            d1 = dpool.tile([128, SC], f32, tag="d")
                            for g, (dd, n) in enumerate(
                                    ((d0, 0), (d0, 1), (d1, 0), (d1, 1))):
                                qrow = slice(32 * g, 32 * (g + 1))
                                sl = slice(n * 512, (n + 1) * 512)
                                ksl = slice(c * SC + n * 512,
                                            c * SC + (n + 1) * 512)
                                nc.tensor.matmul(dd[:, sl], lhsT=qT4[qrow, tsl],
                                                 rhs=kT4[qrow, ksl],
                                                 tile_position=(32 * g, 0))
                            r0 = rpool.tile([128, SC], bf16, tag="r0")
                            nc.scalar.activation(r0[:], d0[:], Act.Relu)
                            r1 = rpool.tile([128, SC], bf16, tag="r1")
                            if plan_b:
                                r1u = rpool.tile([128, SC], bf16, tag="r1u")
                                nc.scalar.activation(r1u[:], d1[:], Act.Relu)
                                nc.vector.tensor_scalar(r1[:], r1u[:], rv, None,
                                                        Alu.mult)
                            else:
                                nc.vector.tensor_scalar(r1[:], d1[:], 0.0, rv,
                                                        Alu.max, Alu.mult)
                            v = rpool.tile([128, SC], bf16, tag="v")
                            nc.vector.tensor_tensor(v[:], r0[:], r1[:], Alu.add)
                            ot = opool.tile([128, SC], out_dt, tag="ot")
                            nc.vector.tensor_scalar(ot[:], v[:], w0, None,
                                                    Alu.mult)
                            nc.sync.dma_start(out[tsl, csl], ot[:])

                    block(0, 0, NJ // 2)
                    proj_half(1)
                    weights_half(1)
                    block(0, NJ // 2, NJ)

                    # chunk-1 keys: contiguous PE block between chunks
                    xs1 = []
                    for kt in range(KT):
                        xs = xs1pool.tile([128, SC], bf16, tag="xs1")
                        nc.sync.dma_start(xs[:], xTs[:, kt, SC:2 * SC])
                        xs1.append(xs)
                    keys_chunk(1, xs1)

                    block(1, 0, NJ)
    nc.compile()
    return nc


def _get_nc():
    if "nc" not in _cached:
        _cached["nc"] = _build()
    return _cached["nc"]


def _make_in_maps(x, wq, wk, ww):
    x_flat = np.asarray(x, dtype=np.float32).reshape(B * T, D_MODEL)
    xT = x_flat.T.astype(BF16)                       # [2048, 8192]
    xTr = np.ascontiguousarray(                      # [128, 16, 8192]
        xT.reshape(KT, 128, B * T).transpose(1, 0, 2))
    wqw = np.ascontiguousarray(
        np.concatenate([np.asarray(wq), np.asarray(ww)], axis=1).astype(BF16)
        .reshape(KT, 128, 66).transpose(1, 0, 2))
    wkk = np.ascontiguousarray(
        np.asarray(wk).astype(BF16).reshape(KT, 128, DIM).transpose(1, 0, 2))
    in_maps = []
    for core in range(N_CORES):
        i, j = core // 2, core % 2
        b = i // 2
        t_lo = i * Q
        s_lo = b * T + j * S
        in_maps.append({
            "xTt": np.ascontiguousarray(xTr[:, :, t_lo:t_lo + Q]),
            "xTs": np.ascontiguousarray(xTr[:, :, s_lo:s_lo + S]),
            "wqw": wqw,
            "wkk": wkk,
        })
    return in_maps


def run(x, wq, wk, ww, trace=False, **kw):
    nc = _get_nc()
    in_maps = _make_in_maps(x, wq, wk, ww)
    res = run_bass_kernel_spmd(nc, in_maps, list(range(N_CORES)),
                               trace=trace, **kw)
    out = np.empty((B * T, T), dtype=np.float32)
    for core in range(N_CORES):
        i, j = core // 2, core % 2
        blk = res.results[core]["out"]
        out[i * Q:(i + 1) * Q, j * S:(j + 1) * S] = blk.astype(np.float32)
    return out.reshape(B, T, T), res


def kernel(x, wq, wk, ww):
    out, _ = run(x, wq, wk, ww, trace=False)
    return out
